# revision 42
# baseline (speedup 1.0000x reference)
"""MoE feed-forward (noisy top-2 gating over 64 experts) on 8 TRN2 NeuronCores.

Strategy (two device phases, host does only the 64-way top-2 bookkeeping):
  Phase 1 (device): tokens sharded 2048/core. Each core computes its shard's
    gate logits  x @ [gate_w | gate_noise_w]  in bf16 (f32 PSUM accumulate),
    applies softplus/noise, and reduces over its tokens -> [64] partial sums.
  Host: sums the 8 partials -> mean logits, top-2 + softmax (matches
    jax.lax.top_k tie semantics via stable argsort), slices + bf16-casts the
    two selected experts' weight tables.
  Phase 2 (device): per core, hT = relu(g_e * (x @ Wi_e)) for both experts
    (gates folded into the relu scale so both experts accumulate into one
    PSUM group), out = sum_e hT_e^T(.)@ Wo_e + sum_e g_e*bo_e, token-sharded.

All matmuls run in bf16 with fp32 PSUM accumulation (measured end-to-end
rel err ~3e-3 vs the fp32 reference; top-2 selection margin is ~4000x the
bf16 gate error on the reference input distribution).

Biases are handled exactly: bi enters the relu as a per-partition bias AP
pre-scaled by the gate on the host; bo enters FFN2 as a rank-1 matmul
(ones^T @ (g0*bo_e0 + g1*bo_e1)).
"""

import sys

for _p in ("/opt/trn_rl_repo", "/root/.axon_site/_ro/trn_rl_repo"):
    if _p not in sys.path:
        sys.path.insert(0, _p)

import ml_dtypes
import numpy as np

import concourse.bass as bass
import concourse.mybir as mybir
import concourse.tile as tile
from concourse.bass_utils import run_bass_kernel_spmd


def _ensure_ntff_hook():
    """Make trace=True / BASS_TRACE profiling work even when the image's
    antenv package lacks axon_hooks (boot then skips hook registration).
    Synthesizes the module and registers the ctypes NTFF hook directly."""
    try:
        import antenv.axon_hooks  # noqa: F401
        return
    except ImportError:
        pass
    try:
        import types

        import antenv

        mod = types.ModuleType("antenv.axon_hooks")
        mod._hook = None

        def set_axon_ntff_profile_hook(hook):
            mod._hook = hook

        def get_axon_ntff_profile_hook():
            return mod._hook

        mod.set_axon_ntff_profile_hook = set_axon_ntff_profile_hook
        mod.get_axon_ntff_profile_hook = get_axon_ntff_profile_hook
        sys.modules["antenv.axon_hooks"] = mod
        antenv.axon_hooks = mod
        from trn_agent_boot.trn_boot import _ntff_profile_via_ctypes

        mod._hook = _ntff_profile_via_ctypes("/opt/axon/libaxon_pjrt.so")
    except Exception:
        pass  # profiling degrades gracefully; execution is unaffected


_ensure_ntff_hook()

# ---------------------------------------------------------------- shapes
B, L, D_IN, D_HID, D_OUT = 4, 4096, 1024, 1024, 1024
E, TOPK = 64, 2
N_CORES = 8
T = B * L            # 16384 tokens
TC = T // N_CORES    # 2048 tokens per core
CH = 512             # token chunk (matmul moving free dim)
NCH = TC // CH       # 4 chunks per core
KB = D_IN // 128     # 8 contraction blocks
HB = D_HID // 128    # 8 hidden blocks

F32 = mybir.dt.float32
BF16 = mybir.dt.bfloat16

# ------------------------------------------------- walrus workaround
# The walrus build in this container supports only ONE sync-wait command
# per instruction; Tile attaches multi-wait lists.  Split them: the tail
# drain via a patched _drain_and_barrier, everything else via a BIR
# post-pass inserting single-wait NoOps ahead of multi-wait instructions.
_TILE_PATCHED = False


def _patch_tile_drain():
    global _TILE_PATCHED
    if _TILE_PATCHED:
        return
    _TILE_PATCHED = True

    def _drain_and_barrier(self, tick_clock, wait_clock):
        n1 = self.nc.sync.nop(nofuse=True)
        wait_clock.add_sem_waits(
            n1.ins, tile.ScopedClock({None: tick_clock.global_clock})
        )
        waits = list(n1.ins.sync_info.on_wait) if n1.ins.sync_info else []
        if len(waits) > 1:
            n1.ins.sync_info.on_wait = waits[:1]
            for i in range(1, len(waits)):
                nx = self.nc.sync.nop(nofuse=True)
                nx.ins.sync_info = mybir.SyncInfo(on_wait=[waits[i]], on_update=[])
        self.nc.sync.drain()
        self.nc.all_engine_barrier()
        assert self.sems is not None
        popped = self.nc._tile_sem_poison_stack.pop()
        assert popped is self._sem_poison
        self.nc.clear_and_free_semaphores(list(self.sems.allocated().values()))
        self.nc.all_engine_barrier()

    tile.TileContext._drain_and_barrier = _drain_and_barrier


def _split_multi_waits(nc):
    n_split = 0
    for f in nc.m.functions:
        for bb in f.blocks:
            insts = list(bb.instructions)
            out = []
            for inst in insts:
                si = inst.sync_info
                if si is not None and si.on_wait and len(si.on_wait) > 1:
                    waits = list(si.on_wait)
                    for w in waits[:-1]:
                        nop = mybir.InstNoOp(
                            name=f"{inst.name}-ws{n_split}", ins=[], outs=[]
                        )
                        nop.engine = inst.engine
                        nop.sync_info = mybir.SyncInfo(on_wait=[w], on_update=[])
                        out.append(nop)
                        n_split += 1
                    si.on_wait = waits[-1:]
                out.append(inst)
            if len(out) != len(insts):
                bb.instructions[:] = out
    return n_split


# ------------------------------------------------------------ builders
def _build_phase1():
    """Gate partials: per core [64,1] f32 = sum over its 2048 tokens of
    x@gate_w + softplus(x@gate_noise_w)*noise   (bf16 matmul, f32 psum)."""
    _patch_tile_drain()
    nc = bass.Bass("TRN2", target_bir_lowering=False, debug=False,
                   num_devices=N_CORES)
    xt_in = nc.dram_tensor("xt", [D_IN, TC], BF16, kind="ExternalInput")
    gw = nc.dram_tensor("gw", [128, KB * 128], BF16, kind="ExternalInput")
    noise = nc.dram_tensor("noise", [E, 1], F32, kind="ExternalInput")
    part = nc.dram_tensor("part", [E, 1], F32, kind="ExternalOutput")

    with tile.TileContext(nc) as tc:
        with (
            tc.tile_pool(name="const", bufs=1) as const,
            tc.tile_pool(name="xt", bufs=1) as xtp,
            tc.tile_pool(name="ps", bufs=1, space="PSUM") as psp,
            tc.tile_pool(name="psw", bufs=1, space="PSUM") as pswp,
            tc.tile_pool(name="sb", bufs=3) as sbp,
            tc.tile_pool(name="red", bufs=NCH + 2) as redp,
        ):
            # PE warmup: ~4.5us of dummy matmuls flips the HAM clock gate
            # to 8/8 while the DMAs stage, so real matmuls run at 2.4GHz.
            wz = const.tile([128, 512], BF16, tag="warm")
            nc.vector.memset(wz[:], 0.0)
            pw = pswp.tile([128, 128], F32, space="PSUM")
            for i in range(40):
                nc.tensor.matmul(pw[:], lhsT=wz[:, :128], rhs=wz[:, :128],
                                 start=(i == 0), stop=(i == 39))

            gw_sb = const.tile([128, KB * 128], BF16)
            nc.scalar.dma_start(out=gw_sb[:], in_=gw[:])
            noise_sb = const.tile([E, 1], F32)
            nc.scalar.dma_start(out=noise_sb[:], in_=noise[:])

            # x pre-transposed; load chunk-major so each chunk's matmuls
            # and softplus tail pipeline behind the next chunk's DMA
            xt_re = xt_in.rearrange("(db p) t -> p db t", p=128)
            xt_chunks = []
            for c in range(NCH):
                xc = xtp.tile([128, KB, CH], BF16, tag=f"xc{c}",
                              name=f"xc{c}")
                nc.sync.dma_start(
                    out=xc[:], in_=xt_re[:, :, c * CH:(c + 1) * CH]
                )
                xt_chunks.append(xc)

            partials = []
            for c in range(NCH):
                ps_g = psp.tile([128, CH], F32, space="PSUM", tag="g",
                                name=f"ps_g{c}", bufs=2)
                for db in range(KB):
                    nc.tensor.matmul(
                        ps_g[:], lhsT=gw_sb[:, db * 128:(db + 1) * 128],
                        rhs=xt_chunks[c][:, db, :],
                        start=(db == 0), stop=(db == KB - 1),
                    )
                # softplus(v) = ln(exp(v) + 1) — this walrus's ACT tables
                # have no native softplus; exp/ln share one func set.
                # Gate pre-activations are O(10), so exp cannot overflow.
                ex = sbp.tile([E, CH], F32)
                nc.scalar.activation(
                    ex[:], ps_g[E:2 * E, :], mybir.ActivationFunctionType.Exp,
                )
                sp = sbp.tile([E, CH], F32)
                nc.scalar.activation(
                    sp[:], ex[:], mybir.ActivationFunctionType.Ln, bias=1.0,
                )
                comb = sbp.tile([E, CH], F32)
                pc = redp.tile([E, 1], F32, tag="partial")
                nc.vector.scalar_tensor_tensor(
                    out=comb[:], in0=sp[:], scalar=noise_sb[:, :1],
                    in1=ps_g[:E, :],
                    op0=mybir.AluOpType.mult, op1=mybir.AluOpType.add,
                    accum_out=pc[:],
                )
                partials.append(pc)
            while len(partials) > 1:
                nxt = []
                for i in range(0, len(partials) - 1, 2):
                    s = redp.tile([E, 1], F32, tag="sum")
                    nc.vector.tensor_add(s[:], partials[i][:], partials[i + 1][:])
                    nxt.append(s)
                if len(partials) % 2:
                    nxt.append(partials[-1])
                partials = nxt
            nc.sync.dma_start(out=part[:], in_=partials[0][:])

    _split_multi_waits(nc)
    return nc


def _build_phase2(with_bo):
    """FFN over the two selected experts, token-sharded, gates folded in.

    DMA layout: x transposes go on the Sync-engine HWDGE FIFO, the 8MB of
    expert weights on the Scalar-engine HWDGE FIFO (separate FIFOs drain
    in parallel across the 16 SDMA engines), wi[0] first so FFN1 matmuls
    start within a few us and the PE HAM warms immediately.
    """
    _patch_tile_drain()
    nc = bass.Bass("TRN2", target_bir_lowering=False, debug=False,
                   num_devices=N_CORES)
    xt_in = nc.dram_tensor("xt", [D_IN, TC], BF16, kind="ExternalInput")
    # host-contiguous layouts: row p holds every block's slice for that
    # partition, so each load is 128 long contiguous descriptors
    wi = nc.dram_tensor("wi", [TOPK, 128, KB * D_HID], BF16,
                        kind="ExternalInput")
    wo = nc.dram_tensor("wo", [TOPK, 128, HB * D_OUT], BF16,
                        kind="ExternalInput")
    scales = nc.dram_tensor("scales", [128, TOPK], F32, kind="ExternalInput")
    bias1 = nc.dram_tensor("bias1", [128, TOPK * HB], F32, kind="ExternalInput")
    if with_bo:
        bo_g = nc.dram_tensor("bo_g", [1, D_OUT], BF16, kind="ExternalInput")
    out = nc.dram_tensor("out", [TC, D_OUT], F32, kind="ExternalOutput")

    with tile.TileContext(nc) as tc:
        with (
            tc.tile_pool(name="const", bufs=1) as const,
            tc.tile_pool(name="xt", bufs=1) as xtp,
            tc.tile_pool(name="psh", bufs=4, space="PSUM") as psh,
            tc.tile_pool(name="pso", bufs=3, space="PSUM") as pso,
            tc.tile_pool(name="psw", bufs=1, space="PSUM") as pswp,
            tc.tile_pool(name="ht", bufs=NCH) as htp,
            tc.tile_pool(name="ob", bufs=3) as obp,
        ):
            # PE warmup while DMAs stage (HAM -> 8/8 before real matmuls)
            wz = const.tile([128, 512], BF16, tag="warm")
            nc.vector.memset(wz[:], 0.0)
            pw = pswp.tile([128, 128], F32, space="PSUM")
            for i in range(44):
                nc.tensor.matmul(pw[:], lhsT=wz[:, :128], rhs=wz[:, :128],
                                 start=(i == 0), stop=(i == 43))

            # Stage in PE-consumption order, split across the two HWDGE
            # FIFOs.  Tiny tensors (relu scale/bias) go first — the first
            # relu needs them at ~16us and anything queued behind the 8MB
            # of weights would land ~50us in.
            scales_sb = const.tile([128, TOPK], F32)
            nc.scalar.dma_start(out=scales_sb[:], in_=scales[:])
            bias1_sb = const.tile([128, TOPK * HB], F32)
            nc.scalar.dma_start(out=bias1_sb[:], in_=bias1[:])
            if with_bo:
                bo_sb = const.tile([1, D_OUT], BF16)
                nc.scalar.dma_start(out=bo_sb[:], in_=bo_g[:])
                ones_sb = const.tile([1, 128], BF16)
                nc.vector.memset(ones_sb[:], 1.0)

            # xt chunk-major on Sync; wi[0] in 8 db-parts on Scalar so the
            # PE streams behind the DMA at ~matching work-per-byte
            xt_re = xt_in.rearrange("(db p) t -> p db t", p=128)
            xt_chunks = []
            for c in range(NCH):
                xc = xtp.tile([128, KB, CH], BF16, tag=f"xc{c}",
                              name=f"xc{c}")
                nc.sync.dma_start(
                    out=xc[:], in_=xt_re[:, :, c * CH:(c + 1) * CH]
                )
                xt_chunks.append(xc)
            wi0_parts = []
            for q in range(KB):
                wq = const.tile([128, D_HID], BF16, tag=f"wi0q{q}",
                                name=f"wi0q{q}")
                nc.scalar.dma_start(
                    out=wq[:], in_=wi[0, :, q * D_HID:(q + 1) * D_HID],
                )
                wi0_parts.append(wq)
            wi1_sb = const.tile([128, KB * D_HID], BF16)
            nc.scalar.dma_start(out=wi1_sb[:], in_=wi[1])
            wo0_sb = const.tile([128, HB * D_OUT], BF16)
            nc.scalar.dma_start(out=wo0_sb[:], in_=wo[0])
            wo1_sb = const.tile([128, HB * D_OUT], BF16)
            nc.scalar.dma_start(out=wo1_sb[:], in_=wo[1])
            wo_sb = [wo0_sb, wo1_sb]

            def wi_lhsT(e, db, h):
                if e == 0:
                    return wi0_parts[db][:, h * 128:(h + 1) * 128]
                return wi1_sb[:, db * D_HID + h * 128:
                              db * D_HID + (h + 1) * 128]

            def xt_rhs(db, c):
                return xt_chunks[c][:, db, :]
            ht_tiles = {}

            def ffn1_e(c, e, db_outer=False):
                # hT[e,h] = relu(g_e * (x @ Wi_e))^T  [dh=128, CH]
                if c not in ht_tiles:
                    ht_tiles[c] = htp.tile([128, TOPK * HB, CH], BF16,
                                           tag="ht", name=f"ht{c}")
                ht = ht_tiles[c]

                def relu_out(h, ph):
                    nc.scalar.activation(
                        ht[:, e * HB + h, :], ph[:],
                        mybir.ActivationFunctionType.Relu,
                        bias=bias1_sb[:, e * HB + h:e * HB + h + 1],
                        scale=scales_sb[:, e:e + 1],
                    )

                if db_outer:
                    # startup shape: 4 h-groups live, db advances outer —
                    # the PE consumes each wi part the moment it lands
                    for h0 in range(0, HB, 4):
                        phs = [
                            psh.tile([128, CH], F32, space="PSUM", tag="ph",
                                     name=f"ph{c}_{e}_{h0 + j}")
                            for j in range(4)
                        ]
                        for db in range(KB):
                            for j in range(4):
                                nc.tensor.matmul(
                                    phs[j][:],
                                    lhsT=wi_lhsT(e, db, h0 + j),
                                    rhs=xt_rhs(db, c),
                                    start=(db == 0), stop=(db == KB - 1),
                                )
                        for j in range(4):
                            relu_out(h0 + j, phs[j])
                    return
                for h in range(HB):
                    ph = psh.tile([128, CH], F32, space="PSUM",
                                  tag="ph", name=f"ph{c}_{e}_{h}")
                    for db in range(KB):
                        nc.tensor.matmul(
                            ph[:],
                            lhsT=wi_lhsT(e, db, h),
                            rhs=xt_rhs(db, c),
                            start=(db == 0), stop=(db == KB - 1),
                        )
                    relu_out(h, ph)

            def ffn2(c):
                # out[tok,do] = sum_{e,h} hT^T @ Wo (+ ones^T @ bo_g)
                ht = ht_tiles.pop(c)
                for tk in range(CH // 128):
                    ob = obp.tile([128, D_OUT], F32, tag="ob",
                                  name=f"ob{c}_{tk}")
                    for n in range(D_OUT // 512):
                        po = pso.tile([128, 512], F32, space="PSUM",
                                      tag="po", name=f"po{c}_{tk}_{n}")
                        n_mm = TOPK * HB
                        k = 0
                        for e in range(TOPK):
                            for h in range(HB):
                                k += 1
                                nc.tensor.matmul(
                                    po[:],
                                    lhsT=ht[:, e * HB + h,
                                            tk * 128:(tk + 1) * 128],
                                    rhs=wo_sb[e][:, h * D_OUT + n * 512:
                                                 h * D_OUT + (n + 1) * 512],
                                    start=(k == 1),
                                    stop=(not with_bo and k == n_mm),
                                )
                        if with_bo:
                            nc.tensor.matmul(
                                po[:], lhsT=ones_sb[:],
                                rhs=bo_sb[:, n * 512:(n + 1) * 512],
                                start=False, stop=True,
                            )
                        nc.vector.tensor_copy(ob[:, n * 512:(n + 1) * 512], po[:])
                    row = c * CH + tk * 128
                    if c == NCH - 1 and tk == CH // 128 - 1:
                        # split the very last store so its first half
                        # overlaps the second half's psum copy
                        nc.sync.dma_start(out=out[row:row + 128, :512],
                                          in_=ob[:, :512])
                        nc.sync.dma_start(out=out[row:row + 128, 512:],
                                          in_=ob[:, 512:])
                    else:
                        nc.sync.dma_start(out=out[row:row + 128, :], in_=ob[:])

            # Pipeline matched to DMA arrival: all e0 FFN1 passes need only
            # x + wi[0] (~54us of PE work), wi[1] lands well before the e1
            # passes, wo before the first FFN2.
            ffn1_e(0, 0, db_outer=True)
            for c in range(1, NCH):
                ffn1_e(c, 0)
            for c in range(NCH):
                ffn1_e(c, 1)
            for c in range(NCH):
                ffn2(c)

    _split_multi_waits(nc)
    return nc


_CACHE = {}


def _phase(name, *args):
    key = (name, *args)
    if key not in _CACHE:
        _CACHE[key] = _build_phase1() if name == "p1" else _build_phase2(*args)
    return _CACHE[key]


def _bf16(a):
    return np.asarray(a, np.float32).astype(ml_dtypes.bfloat16)


def kernel(x, noise, gate_w, gate_noise_w, Wi, bi, Wo, bo, _timing=None):
    x = np.asarray(x, np.float32)
    noise = np.asarray(noise, np.float32)
    gate_w = np.asarray(gate_w, np.float32)
    gate_noise_w = np.asarray(gate_noise_w, np.float32)
    bi = np.asarray(bi, np.float32)
    bo = np.asarray(bo, np.float32)

    xb = _bf16(x.reshape(T, D_IN))
    # host-side transpose: device loads xT with plain contiguous DMAs
    # (the on-chip alternatives — xbar DMA-transpose or PE transposes —
    # measured ~2x slower than line-rate and serialized kernel startup)
    xt_shards = [
        np.ascontiguousarray(xb[c * TC:(c + 1) * TC].T) for c in range(N_CORES)
    ]
    core_ids = list(range(N_CORES))

    # ---- phase 1: gate partials
    gw_cat = _bf16(np.concatenate([gate_w, gate_noise_w], axis=1))
    # [p, db*128+e] layout: one contiguous row per partition
    gw_host = np.ascontiguousarray(
        gw_cat.reshape(KB, 128, 128).transpose(1, 0, 2).reshape(128, KB * 128)
    )
    noise_col = noise.reshape(E, 1)
    in1 = [
        {"xt": xt_shards[c], "gw": gw_host, "noise": noise_col}
        for c in range(N_CORES)
    ]
    r1 = run_bass_kernel_spmd(_phase("p1"), in1, core_ids,
                              **(_timing or {}).get("p1", {}))
    mean_logits = (
        sum(r1.results[c]["part"][:, 0].astype(np.float64)
            for c in range(N_CORES)) / T
    ).astype(np.float32)

    # ---- host routing: top-2 + softmax (stable => jax.lax.top_k ties)
    idx = np.argsort(-mean_logits, kind="stable")[:TOPK]
    tv = mean_logits[idx]
    ex = np.exp(tv - tv.max())
    gates = (ex / ex.sum()).astype(np.float32)

    # ---- phase 2: FFN on the two selected experts
    # [e, p, db*D + col] layout: one contiguous row per partition
    wi_sel = np.ascontiguousarray(
        _bf16(np.asarray(Wi)[idx]).reshape(TOPK, KB, 128, D_HID)
        .transpose(0, 2, 1, 3).reshape(TOPK, 128, KB * D_HID)
    )
    wo_sel = np.ascontiguousarray(
        _bf16(np.asarray(Wo)[idx]).reshape(TOPK, HB, 128, D_OUT)
        .transpose(0, 2, 1, 3).reshape(TOPK, 128, HB * D_OUT)
    )
    scales = np.broadcast_to(gates, (128, TOPK)).copy()
    # bias1[p, e*HB+h] = g_e * bi[e_sel, h*128+p]
    bias1 = (gates[:, None] * bi[idx]).reshape(TOPK, HB, 128)
    bias1 = np.ascontiguousarray(bias1.transpose(2, 0, 1).reshape(128, TOPK * HB))
    with_bo = bool(np.any(bo[idx]))
    in2 = [
        {
            "xt": xt_shards[c], "wi": wi_sel, "wo": wo_sel,
            "scales": scales, "bias1": bias1,
        }
        for c in range(N_CORES)
    ]
    if with_bo:
        bo_g = _bf16((gates[:, None] * bo[idx]).sum(0).reshape(1, D_OUT))
        for m in in2:
            m["bo_g"] = bo_g
    r2 = run_bass_kernel_spmd(_phase("p2", with_bo), in2, core_ids,
                              **(_timing or {}).get("p2", {}))
    out = np.concatenate([r2.results[c]["out"] for c in range(N_CORES)], axis=0)

    if isinstance(_timing, dict):
        _timing["exec_ns"] = [r1.exec_time_ns, r2.exec_time_ns]
    return out.reshape(B, L, D_OUT).astype(np.float32, copy=False)


# revision 43
# speedup vs baseline: 1.1800x; 1.1800x over previous
"""MoE feed-forward (noisy top-2 gating over 64 experts) on 8 TRN2 NeuronCores.

Strategy (two device phases, host does only the 64-way top-2 bookkeeping):
  Phase 1 (device): tokens sharded 2048/core. Each core computes its shard's
    gate logits  x @ [gate_w | gate_noise_w]  in bf16 (f32 PSUM accumulate),
    applies softplus/noise, and reduces over its tokens -> [64] partial sums.
  Host: sums the 8 partials -> mean logits, top-2 + softmax (matches
    jax.lax.top_k tie semantics via stable argsort), slices + bf16-casts the
    two selected experts' weight tables.
  Phase 2 (device): per core, hT = relu(g_e * (x @ Wi_e)) for both experts
    (gates folded into the relu scale so both experts accumulate into one
    PSUM group), out = sum_e hT_e^T(.)@ Wo_e + sum_e g_e*bo_e, token-sharded.

All matmuls run in bf16 with fp32 PSUM accumulation (measured end-to-end
rel err ~3e-3 vs the fp32 reference; top-2 selection margin is ~4000x the
bf16 gate error on the reference input distribution).

Biases are handled exactly: bi enters the relu as a per-partition bias AP
pre-scaled by the gate on the host; bo enters FFN2 as a rank-1 matmul
(ones^T @ (g0*bo_e0 + g1*bo_e1)).
"""

import sys

for _p in ("/opt/trn_rl_repo", "/root/.axon_site/_ro/trn_rl_repo"):
    if _p not in sys.path:
        sys.path.insert(0, _p)

import ml_dtypes
import numpy as np

import concourse.bass as bass
import concourse.mybir as mybir
import concourse.tile as tile
from concourse.bass_utils import run_bass_kernel_spmd


def _ensure_ntff_hook():
    """Make trace=True / BASS_TRACE profiling work even when the image's
    antenv package lacks axon_hooks (boot then skips hook registration).
    Synthesizes the module and registers the ctypes NTFF hook directly."""
    try:
        import antenv.axon_hooks  # noqa: F401
        return
    except ImportError:
        pass
    try:
        import types

        import antenv

        mod = types.ModuleType("antenv.axon_hooks")
        mod._hook = None

        def set_axon_ntff_profile_hook(hook):
            mod._hook = hook

        def get_axon_ntff_profile_hook():
            return mod._hook

        mod.set_axon_ntff_profile_hook = set_axon_ntff_profile_hook
        mod.get_axon_ntff_profile_hook = get_axon_ntff_profile_hook
        sys.modules["antenv.axon_hooks"] = mod
        antenv.axon_hooks = mod
        from trn_agent_boot.trn_boot import _ntff_profile_via_ctypes

        mod._hook = _ntff_profile_via_ctypes("/opt/axon/libaxon_pjrt.so")
    except Exception:
        pass  # profiling degrades gracefully; execution is unaffected


_ensure_ntff_hook()

# ---------------------------------------------------------------- shapes
B, L, D_IN, D_HID, D_OUT = 4, 4096, 1024, 1024, 1024
E, TOPK = 64, 2
N_CORES = 8
T = B * L            # 16384 tokens
TC = T // N_CORES    # 2048 tokens per core
CH = 512             # token chunk (matmul moving free dim)
NCH = TC // CH       # 4 chunks per core
KB = D_IN // 128     # 8 contraction blocks
HB = D_HID // 128    # 8 hidden blocks

F32 = mybir.dt.float32
BF16 = mybir.dt.bfloat16

# ------------------------------------------------- walrus workaround
# The walrus build in this container supports only ONE sync-wait command
# per instruction; Tile attaches multi-wait lists.  Split them: the tail
# drain via a patched _drain_and_barrier, everything else via a BIR
# post-pass inserting single-wait NoOps ahead of multi-wait instructions.
_TILE_PATCHED = False


def _patch_tile_drain():
    global _TILE_PATCHED
    if _TILE_PATCHED:
        return
    _TILE_PATCHED = True

    def _drain_and_barrier(self, tick_clock, wait_clock):
        n1 = self.nc.sync.nop(nofuse=True)
        wait_clock.add_sem_waits(
            n1.ins, tile.ScopedClock({None: tick_clock.global_clock})
        )
        waits = list(n1.ins.sync_info.on_wait) if n1.ins.sync_info else []
        if len(waits) > 1:
            n1.ins.sync_info.on_wait = waits[:1]
            for i in range(1, len(waits)):
                nx = self.nc.sync.nop(nofuse=True)
                nx.ins.sync_info = mybir.SyncInfo(on_wait=[waits[i]], on_update=[])
        self.nc.sync.drain()
        self.nc.all_engine_barrier()
        assert self.sems is not None
        popped = self.nc._tile_sem_poison_stack.pop()
        assert popped is self._sem_poison
        self.nc.clear_and_free_semaphores(list(self.sems.allocated().values()))
        self.nc.all_engine_barrier()

    tile.TileContext._drain_and_barrier = _drain_and_barrier


def _split_multi_waits(nc):
    n_split = 0
    for f in nc.m.functions:
        for bb in f.blocks:
            insts = list(bb.instructions)
            out = []
            for inst in insts:
                si = inst.sync_info
                if si is not None and si.on_wait and len(si.on_wait) > 1:
                    waits = list(si.on_wait)
                    for w in waits[:-1]:
                        nop = mybir.InstNoOp(
                            name=f"{inst.name}-ws{n_split}", ins=[], outs=[]
                        )
                        nop.engine = inst.engine
                        nop.sync_info = mybir.SyncInfo(on_wait=[w], on_update=[])
                        out.append(nop)
                        n_split += 1
                    si.on_wait = waits[-1:]
                out.append(inst)
            if len(out) != len(insts):
                bb.instructions[:] = out
    return n_split


# ------------------------------------------------------------ builders
def _build_phase1():
    """Gate partials: per core [64,1] f32 = sum over its 2048 tokens of
    x@gate_w + softplus(x@gate_noise_w)*noise   (bf16 matmul, f32 psum)."""
    _patch_tile_drain()
    nc = bass.Bass("TRN2", target_bir_lowering=False, debug=False,
                   num_devices=N_CORES)
    xt_in = nc.dram_tensor("xt", [D_IN, TC], BF16, kind="ExternalInput")
    gw = nc.dram_tensor("gw", [128, KB * 128], BF16, kind="ExternalInput")
    noise = nc.dram_tensor("noise", [E, 1], F32, kind="ExternalInput")
    part = nc.dram_tensor("part", [E, 1], F32, kind="ExternalOutput")

    with tile.TileContext(nc) as tc:
        with (
            tc.tile_pool(name="const", bufs=1) as const,
            tc.tile_pool(name="xt", bufs=1) as xtp,
            tc.tile_pool(name="ps", bufs=1, space="PSUM") as psp,
            tc.tile_pool(name="psw", bufs=1, space="PSUM") as pswp,
            tc.tile_pool(name="sb", bufs=3) as sbp,
            tc.tile_pool(name="red", bufs=NCH + 2) as redp,
        ):
            # PE warmup: ~4.5us of dummy matmuls flips the HAM clock gate
            # to 8/8 while the DMAs stage, so real matmuls run at 2.4GHz.
            wz = const.tile([128, 512], BF16, tag="warm")
            nc.vector.memset(wz[:], 0.0)
            pw = pswp.tile([128, 128], F32, space="PSUM")
            for i in range(40):
                nc.tensor.matmul(pw[:], lhsT=wz[:, :128], rhs=wz[:, :128],
                                 start=(i == 0), stop=(i == 39))

            gw_sb = const.tile([128, KB * 128], BF16)
            nc.scalar.dma_start(out=gw_sb[:], in_=gw[:])
            noise_sb = const.tile([E, 1], F32)
            nc.scalar.dma_start(out=noise_sb[:], in_=noise[:])

            # x pre-transposed; load chunk-major so each chunk's matmuls
            # and softplus tail pipeline behind the next chunk's DMA
            xt_re = xt_in.rearrange("(db p) t -> p db t", p=128)
            xt_chunks = []
            for c in range(NCH):
                xc = xtp.tile([128, KB, CH], BF16, tag=f"xc{c}",
                              name=f"xc{c}")
                nc.sync.dma_start(
                    out=xc[:], in_=xt_re[:, :, c * CH:(c + 1) * CH]
                )
                xt_chunks.append(xc)

            partials = []
            for c in range(NCH):
                ps_g = psp.tile([128, CH], F32, space="PSUM", tag="g",
                                name=f"ps_g{c}", bufs=2)
                for db in range(KB):
                    nc.tensor.matmul(
                        ps_g[:], lhsT=gw_sb[:, db * 128:(db + 1) * 128],
                        rhs=xt_chunks[c][:, db, :],
                        start=(db == 0), stop=(db == KB - 1),
                    )
                # softplus(v) = ln(exp(v) + 1) — this walrus's ACT tables
                # have no native softplus; exp/ln share one func set.
                # Gate pre-activations are O(10), so exp cannot overflow.
                ex = sbp.tile([E, CH], F32)
                nc.scalar.activation(
                    ex[:], ps_g[E:2 * E, :], mybir.ActivationFunctionType.Exp,
                )
                sp = sbp.tile([E, CH], F32)
                nc.scalar.activation(
                    sp[:], ex[:], mybir.ActivationFunctionType.Ln, bias=1.0,
                )
                comb = sbp.tile([E, CH], F32)
                pc = redp.tile([E, 1], F32, tag="partial")
                nc.vector.scalar_tensor_tensor(
                    out=comb[:], in0=sp[:], scalar=noise_sb[:, :1],
                    in1=ps_g[:E, :],
                    op0=mybir.AluOpType.mult, op1=mybir.AluOpType.add,
                    accum_out=pc[:],
                )
                partials.append(pc)
            while len(partials) > 1:
                nxt = []
                for i in range(0, len(partials) - 1, 2):
                    s = redp.tile([E, 1], F32, tag="sum")
                    nc.vector.tensor_add(s[:], partials[i][:], partials[i + 1][:])
                    nxt.append(s)
                if len(partials) % 2:
                    nxt.append(partials[-1])
                partials = nxt
            nc.sync.dma_start(out=part[:], in_=partials[0][:])

    _split_multi_waits(nc)
    return nc


def _build_phase2(with_bo):
    """FFN over the two selected experts, token-sharded, gates folded in.

    DMA layout: pre-transposed x loads on the Sync-engine HWDGE FIFO, the
    8MB of expert weights on the Scalar-engine HWDGE FIFO (the FIFOs drain
    in parallel across the 16 SDMA engines), wi[0] first in fine parts so
    FFN1 matmuls start within a few us and the PE HAM stays warm.
    """
    _patch_tile_drain()
    nc = bass.Bass("TRN2", target_bir_lowering=False, debug=False,
                   num_devices=N_CORES)
    xt_in = nc.dram_tensor("xt", [D_IN, TC], BF16, kind="ExternalInput")
    # host-contiguous layouts: row p holds every block's slice for that
    # partition, so each load is 128 long contiguous descriptors
    wi = nc.dram_tensor("wi", [TOPK, 128, KB * D_HID], BF16,
                        kind="ExternalInput")
    wo = nc.dram_tensor("wo", [TOPK, 128, HB * D_OUT], BF16,
                        kind="ExternalInput")
    scales = nc.dram_tensor("scales", [128, TOPK], F32, kind="ExternalInput")
    bias1 = nc.dram_tensor("bias1", [128, TOPK * HB], F32, kind="ExternalInput")
    if with_bo:
        bo_g = nc.dram_tensor("bo_g", [1, D_OUT], BF16, kind="ExternalInput")
    out = nc.dram_tensor("out", [TC, D_OUT], F32, kind="ExternalOutput")

    with tile.TileContext(nc) as tc:
        with (
            tc.tile_pool(name="const", bufs=1) as const,
            tc.tile_pool(name="xt", bufs=1) as xtp,
            tc.tile_pool(name="psh", bufs=4, space="PSUM") as psh,
            tc.tile_pool(name="pso", bufs=3, space="PSUM") as pso,
            tc.tile_pool(name="psw", bufs=1, space="PSUM") as pswp,
            tc.tile_pool(name="ht", bufs=NCH) as htp,
            tc.tile_pool(name="ob", bufs=3) as obp,
        ):
            # PE warmup while DMAs stage (HAM -> 8/8 before real matmuls)
            wz = const.tile([128, 512], BF16, tag="warm")
            nc.vector.memset(wz[:], 0.0)
            pw = pswp.tile([128, 128], F32, space="PSUM")
            for i in range(44):
                nc.tensor.matmul(pw[:], lhsT=wz[:, :128], rhs=wz[:, :128],
                                 start=(i == 0), stop=(i == 43))

            # Stage in PE-consumption order, split across the two HWDGE
            # FIFOs.  Tiny tensors (relu scale/bias) go first — the first
            # relu needs them at ~16us and anything queued behind the 8MB
            # of weights would land ~50us in.
            scales_sb = const.tile([128, TOPK], F32)
            nc.scalar.dma_start(out=scales_sb[:], in_=scales[:])
            bias1_sb = const.tile([128, TOPK * HB], F32)
            nc.scalar.dma_start(out=bias1_sb[:], in_=bias1[:])
            if with_bo:
                bo_sb = const.tile([1, D_OUT], BF16)
                nc.scalar.dma_start(out=bo_sb[:], in_=bo_g[:])
                ones_sb = const.tile([1, 128], BF16)
                nc.vector.memset(ones_sb[:], 1.0)

            # xt chunk-major on Sync; wi[0] in 8 db-parts on Scalar so the
            # PE streams behind the DMA at ~matching work-per-byte
            xt_re = xt_in.rearrange("(db p) t -> p db t", p=128)
            xt_chunks = []
            for c in range(NCH):
                xc = xtp.tile([128, KB, CH], BF16, tag=f"xc{c}",
                              name=f"xc{c}")
                nc.sync.dma_start(
                    out=xc[:], in_=xt_re[:, :, c * CH:(c + 1) * CH]
                )
                xt_chunks.append(xc)
            wi0_parts = []
            for q in range(KB):
                wq = const.tile([128, D_HID], BF16, tag=f"wi0q{q}",
                                name=f"wi0q{q}")
                nc.scalar.dma_start(
                    out=wq[:], in_=wi[0, :, q * D_HID:(q + 1) * D_HID],
                )
                wi0_parts.append(wq)
            wi1_sb = const.tile([128, KB * D_HID], BF16)
            nc.scalar.dma_start(out=wi1_sb[:], in_=wi[1])
            wo0_sb = const.tile([128, HB * D_OUT], BF16)
            nc.scalar.dma_start(out=wo0_sb[:], in_=wo[0])
            wo1_sb = const.tile([128, HB * D_OUT], BF16)
            nc.scalar.dma_start(out=wo1_sb[:], in_=wo[1])
            wo_sb = [wo0_sb, wo1_sb]

            def wi_lhsT(e, db, h):
                if e == 0:
                    return wi0_parts[db][:, h * 128:(h + 1) * 128]
                return wi1_sb[:, db * D_HID + h * 128:
                              db * D_HID + (h + 1) * 128]

            def xt_rhs(db, c):
                return xt_chunks[c][:, db, :]
            ht_tiles = {}

            def ffn1_e(c, e, db_outer=False):
                # hT[e,h] = relu(g_e * (x @ Wi_e))^T  [dh=128, CH]
                if c not in ht_tiles:
                    ht_tiles[c] = htp.tile([128, TOPK * HB, CH], BF16,
                                           tag="ht", name=f"ht{c}")
                ht = ht_tiles[c]

                def relu_out(h, ph):
                    nc.scalar.activation(
                        ht[:, e * HB + h, :], ph[:],
                        mybir.ActivationFunctionType.Relu,
                        bias=bias1_sb[:, e * HB + h:e * HB + h + 1],
                        scale=scales_sb[:, e:e + 1],
                    )

                if db_outer:
                    # startup shape: 4 h-groups live, db advances outer —
                    # the PE consumes each wi part the moment it lands
                    for h0 in range(0, HB, 4):
                        phs = [
                            psh.tile([128, CH], F32, space="PSUM", tag="ph",
                                     name=f"ph{c}_{e}_{h0 + j}")
                            for j in range(4)
                        ]
                        for db in range(KB):
                            for j in range(4):
                                nc.tensor.matmul(
                                    phs[j][:],
                                    lhsT=wi_lhsT(e, db, h0 + j),
                                    rhs=xt_rhs(db, c),
                                    start=(db == 0), stop=(db == KB - 1),
                                )
                        for j in range(4):
                            relu_out(h0 + j, phs[j])
                    return
                for h in range(HB):
                    ph = psh.tile([128, CH], F32, space="PSUM",
                                  tag="ph", name=f"ph{c}_{e}_{h}")
                    for db in range(KB):
                        nc.tensor.matmul(
                            ph[:],
                            lhsT=wi_lhsT(e, db, h),
                            rhs=xt_rhs(db, c),
                            start=(db == 0), stop=(db == KB - 1),
                        )
                    relu_out(h, ph)

            def ffn2(c):
                # out[tok,do] = sum_{e,h} hT^T @ Wo (+ ones^T @ bo_g)
                ht = ht_tiles.pop(c)
                for tk in range(CH // 128):
                    ob = obp.tile([128, D_OUT], F32, tag="ob",
                                  name=f"ob{c}_{tk}")
                    for n in range(D_OUT // 512):
                        po = pso.tile([128, 512], F32, space="PSUM",
                                      tag="po", name=f"po{c}_{tk}_{n}")
                        n_mm = TOPK * HB
                        k = 0
                        for e in range(TOPK):
                            for h in range(HB):
                                k += 1
                                nc.tensor.matmul(
                                    po[:],
                                    lhsT=ht[:, e * HB + h,
                                            tk * 128:(tk + 1) * 128],
                                    rhs=wo_sb[e][:, h * D_OUT + n * 512:
                                                 h * D_OUT + (n + 1) * 512],
                                    start=(k == 1),
                                    stop=(not with_bo and k == n_mm),
                                )
                        if with_bo:
                            nc.tensor.matmul(
                                po[:], lhsT=ones_sb[:],
                                rhs=bo_sb[:, n * 512:(n + 1) * 512],
                                start=False, stop=True,
                            )
                        nc.vector.tensor_copy(ob[:, n * 512:(n + 1) * 512], po[:])
                    row = c * CH + tk * 128
                    if c == NCH - 1 and tk == CH // 128 - 1:
                        # split the very last store so its first half
                        # overlaps the second half's psum copy
                        nc.sync.dma_start(out=out[row:row + 128, :512],
                                          in_=ob[:, :512])
                        nc.sync.dma_start(out=out[row:row + 128, 512:],
                                          in_=ob[:, 512:])
                    else:
                        nc.sync.dma_start(out=out[row:row + 128, :], in_=ob[:])

            # Pipeline matched to DMA arrival: all e0 FFN1 passes need only
            # x + wi[0] (~54us of PE work), wi[1] lands well before the e1
            # passes, wo before the first FFN2.
            ffn1_e(0, 0, db_outer=True)
            for c in range(1, NCH):
                ffn1_e(c, 0)
            for c in range(NCH):
                ffn1_e(c, 1)
            for c in range(NCH):
                ffn2(c)

    _split_multi_waits(nc)
    return nc


_CACHE = {}


def _phase(name, *args):
    key = (name, *args)
    if key not in _CACHE:
        _CACHE[key] = _build_phase1() if name == "p1" else _build_phase2(*args)
    return _CACHE[key]


def _bf16(a):
    return np.asarray(a, np.float32).astype(ml_dtypes.bfloat16)


def kernel(x, noise, gate_w, gate_noise_w, Wi, bi, Wo, bo, _timing=None):
    x = np.asarray(x, np.float32)
    noise = np.asarray(noise, np.float32)
    gate_w = np.asarray(gate_w, np.float32)
    gate_noise_w = np.asarray(gate_noise_w, np.float32)
    bi = np.asarray(bi, np.float32)
    bo = np.asarray(bo, np.float32)

    xb = _bf16(x.reshape(T, D_IN))
    # host-side transpose: device loads xT with plain contiguous DMAs
    # (the on-chip alternatives — xbar DMA-transpose or PE transposes —
    # measured ~2x slower than line-rate and serialized kernel startup)
    xt_shards = [
        np.ascontiguousarray(xb[c * TC:(c + 1) * TC].T) for c in range(N_CORES)
    ]
    core_ids = list(range(N_CORES))

    # ---- phase 1: gate partials
    gw_cat = _bf16(np.concatenate([gate_w, gate_noise_w], axis=1))
    # [p, db*128+e] layout: one contiguous row per partition
    gw_host = np.ascontiguousarray(
        gw_cat.reshape(KB, 128, 128).transpose(1, 0, 2).reshape(128, KB * 128)
    )
    noise_col = noise.reshape(E, 1)
    in1 = [
        {"xt": xt_shards[c], "gw": gw_host, "noise": noise_col}
        for c in range(N_CORES)
    ]
    r1 = run_bass_kernel_spmd(_phase("p1"), in1, core_ids,
                              **(_timing or {}).get("p1", {}))
    mean_logits = (
        sum(r1.results[c]["part"][:, 0].astype(np.float64)
            for c in range(N_CORES)) / T
    ).astype(np.float32)

    # ---- host routing: top-2 + softmax (stable => jax.lax.top_k ties)
    idx = np.argsort(-mean_logits, kind="stable")[:TOPK]
    tv = mean_logits[idx]
    ex = np.exp(tv - tv.max())
    gates = (ex / ex.sum()).astype(np.float32)

    # ---- phase 2: FFN on the two selected experts
    # [e, p, db*D + col] layout: one contiguous row per partition
    wi_sel = np.ascontiguousarray(
        _bf16(np.asarray(Wi)[idx]).reshape(TOPK, KB, 128, D_HID)
        .transpose(0, 2, 1, 3).reshape(TOPK, 128, KB * D_HID)
    )
    wo_sel = np.ascontiguousarray(
        _bf16(np.asarray(Wo)[idx]).reshape(TOPK, HB, 128, D_OUT)
        .transpose(0, 2, 1, 3).reshape(TOPK, 128, HB * D_OUT)
    )
    scales = np.broadcast_to(gates, (128, TOPK)).copy()
    # bias1[p, e*HB+h] = g_e * bi[e_sel, h*128+p]
    bias1 = (gates[:, None] * bi[idx]).reshape(TOPK, HB, 128)
    bias1 = np.ascontiguousarray(bias1.transpose(2, 0, 1).reshape(128, TOPK * HB))
    with_bo = bool(np.any(bo[idx]))
    in2 = [
        {
            "xt": xt_shards[c], "wi": wi_sel, "wo": wo_sel,
            "scales": scales, "bias1": bias1,
        }
        for c in range(N_CORES)
    ]
    if with_bo:
        bo_g = _bf16((gates[:, None] * bo[idx]).sum(0).reshape(1, D_OUT))
        for m in in2:
            m["bo_g"] = bo_g
    r2 = run_bass_kernel_spmd(_phase("p2", with_bo), in2, core_ids,
                              **(_timing or {}).get("p2", {}))
    out = np.concatenate([r2.results[c]["out"] for c in range(N_CORES)], axis=0)

    if isinstance(_timing, dict):
        _timing["exec_ns"] = [r1.exec_time_ns, r2.exec_time_ns]
    return out.reshape(B, L, D_OUT).astype(np.float32, copy=False)


# revision 49
# speedup vs baseline: 1.1827x; 1.0023x over previous
"""MoE feed-forward (noisy top-2 gating over 64 experts) on 8 TRN2 NeuronCores.

Strategy (two device phases, host does only the 64-way top-2 bookkeeping):
  Phase 1 (device): tokens sharded 2048/core. Each core computes its shard's
    gate logits  x @ [gate_w | gate_noise_w]  in bf16 (f32 PSUM accumulate),
    applies softplus/noise, and reduces over its tokens -> [64] partial sums.
  Host: sums the 8 partials -> mean logits, top-2 + softmax (matches
    jax.lax.top_k tie semantics via stable argsort), slices + bf16-casts the
    two selected experts' weight tables.
  Phase 2 (device): per core, hT = relu(g_e * (x @ Wi_e)) for both experts
    (gates folded into the relu scale so both experts accumulate into one
    PSUM group), out = sum_e hT_e^T(.)@ Wo_e + sum_e g_e*bo_e, token-sharded.

All matmuls run in bf16 with fp32 PSUM accumulation (measured end-to-end
rel err ~3e-3 vs the fp32 reference; top-2 selection margin is ~4000x the
bf16 gate error on the reference input distribution).

Biases are handled exactly: bi enters the relu as a per-partition bias AP
pre-scaled by the gate on the host; bo enters FFN2 as a rank-1 matmul
(ones^T @ (g0*bo_e0 + g1*bo_e1)).
"""

import sys

for _p in ("/opt/trn_rl_repo", "/root/.axon_site/_ro/trn_rl_repo"):
    if _p not in sys.path:
        sys.path.insert(0, _p)

import ml_dtypes
import numpy as np

import concourse.bass as bass
import concourse.mybir as mybir
import concourse.tile as tile
from concourse.bass_utils import run_bass_kernel_spmd


def _ensure_ntff_hook():
    """Make trace=True / BASS_TRACE profiling work even when the image's
    antenv package lacks axon_hooks (boot then skips hook registration).
    Synthesizes the module and registers the ctypes NTFF hook directly."""
    try:
        import antenv.axon_hooks  # noqa: F401
        return
    except ImportError:
        pass
    try:
        import types

        import antenv

        mod = types.ModuleType("antenv.axon_hooks")
        mod._hook = None

        def set_axon_ntff_profile_hook(hook):
            mod._hook = hook

        def get_axon_ntff_profile_hook():
            return mod._hook

        mod.set_axon_ntff_profile_hook = set_axon_ntff_profile_hook
        mod.get_axon_ntff_profile_hook = get_axon_ntff_profile_hook
        sys.modules["antenv.axon_hooks"] = mod
        antenv.axon_hooks = mod
        from trn_agent_boot.trn_boot import _ntff_profile_via_ctypes

        mod._hook = _ntff_profile_via_ctypes("/opt/axon/libaxon_pjrt.so")
    except Exception:
        pass  # profiling degrades gracefully; execution is unaffected


_ensure_ntff_hook()

# ---------------------------------------------------------------- shapes
B, L, D_IN, D_HID, D_OUT = 4, 4096, 1024, 1024, 1024
E, TOPK = 64, 2
N_CORES = 8
T = B * L            # 16384 tokens
TC = T // N_CORES    # 2048 tokens per core
CH = 512             # token chunk (matmul moving free dim)
NCH = TC // CH       # 4 chunks per core
KB = D_IN // 128     # 8 contraction blocks
HB = D_HID // 128    # 8 hidden blocks

F32 = mybir.dt.float32
BF16 = mybir.dt.bfloat16

# ------------------------------------------------- walrus workaround
# The walrus build in this container supports only ONE sync-wait command
# per instruction; Tile attaches multi-wait lists.  Split them: the tail
# drain via a patched _drain_and_barrier, everything else via a BIR
# post-pass inserting single-wait NoOps ahead of multi-wait instructions.
_TILE_PATCHED = False


def _patch_tile_drain():
    global _TILE_PATCHED
    if _TILE_PATCHED:
        return
    _TILE_PATCHED = True

    def _drain_and_barrier(self, tick_clock, wait_clock):
        n1 = self.nc.sync.nop(nofuse=True)
        wait_clock.add_sem_waits(
            n1.ins, tile.ScopedClock({None: tick_clock.global_clock})
        )
        waits = list(n1.ins.sync_info.on_wait) if n1.ins.sync_info else []
        if len(waits) > 1:
            n1.ins.sync_info.on_wait = waits[:1]
            for i in range(1, len(waits)):
                nx = self.nc.sync.nop(nofuse=True)
                nx.ins.sync_info = mybir.SyncInfo(on_wait=[waits[i]], on_update=[])
        self.nc.sync.drain()
        self.nc.all_engine_barrier()
        assert self.sems is not None
        popped = self.nc._tile_sem_poison_stack.pop()
        assert popped is self._sem_poison
        self.nc.clear_and_free_semaphores(list(self.sems.allocated().values()))
        self.nc.all_engine_barrier()

    tile.TileContext._drain_and_barrier = _drain_and_barrier


def _split_multi_waits(nc):
    n_split = 0
    for f in nc.m.functions:
        for bb in f.blocks:
            insts = list(bb.instructions)
            out = []
            for inst in insts:
                si = inst.sync_info
                if si is not None and si.on_wait and len(si.on_wait) > 1:
                    waits = list(si.on_wait)
                    for w in waits[:-1]:
                        nop = mybir.InstNoOp(
                            name=f"{inst.name}-ws{n_split}", ins=[], outs=[]
                        )
                        nop.engine = inst.engine
                        nop.sync_info = mybir.SyncInfo(on_wait=[w], on_update=[])
                        out.append(nop)
                        n_split += 1
                    si.on_wait = waits[-1:]
                out.append(inst)
            if len(out) != len(insts):
                bb.instructions[:] = out
    return n_split


# ------------------------------------------------------------ builders
def _build_phase1():
    """Gate partials: per core [64,1] f32 = sum over its 2048 tokens of
    x@gate_w + softplus(x@gate_noise_w)*noise   (bf16 matmul, f32 psum)."""
    _patch_tile_drain()
    nc = bass.Bass("TRN2", target_bir_lowering=False, debug=False,
                   num_devices=N_CORES)
    xt_in = nc.dram_tensor("xt", [D_IN, TC], BF16, kind="ExternalInput")
    gw = nc.dram_tensor("gw", [128, KB * 128], BF16, kind="ExternalInput")
    noise = nc.dram_tensor("noise", [E, 1], F32, kind="ExternalInput")
    part = nc.dram_tensor("part", [E, 1], F32, kind="ExternalOutput")

    with tile.TileContext(nc) as tc:
        with (
            tc.tile_pool(name="const", bufs=1) as const,
            tc.tile_pool(name="xt", bufs=1) as xtp,
            tc.tile_pool(name="ps", bufs=1, space="PSUM") as psp,
            tc.tile_pool(name="psw", bufs=1, space="PSUM") as pswp,
            tc.tile_pool(name="sb", bufs=3) as sbp,
            tc.tile_pool(name="red", bufs=NCH + 2) as redp,
        ):
            # PE warmup: ~4.5us of dummy matmuls flips the HAM clock gate
            # to 8/8 while the DMAs stage, so real matmuls run at 2.4GHz.
            wz = const.tile([128, 512], BF16, tag="warm")
            nc.vector.memset(wz[:], 0.0)
            pw = pswp.tile([128, 128], F32, space="PSUM")
            for i in range(40):
                nc.tensor.matmul(pw[:], lhsT=wz[:, :128], rhs=wz[:, :128],
                                 start=(i == 0), stop=(i == 39))

            gw_sb = const.tile([128, KB * 128], BF16)
            nc.scalar.dma_start(out=gw_sb[:], in_=gw[:])
            noise_sb = const.tile([E, 1], F32)
            nc.scalar.dma_start(out=noise_sb[:], in_=noise[:])

            # x pre-transposed; load chunk-major so each chunk's matmuls
            # and softplus tail pipeline behind the next chunk's DMA
            xt_re = xt_in.rearrange("(db p) t -> p db t", p=128)
            xt_chunks = []
            for c in range(NCH):
                xc = xtp.tile([128, KB, CH], BF16, tag=f"xc{c}",
                              name=f"xc{c}")
                nc.sync.dma_start(
                    out=xc[:], in_=xt_re[:, :, c * CH:(c + 1) * CH]
                )
                xt_chunks.append(xc)

            partials = []
            for c in range(NCH):
                ps_g = psp.tile([128, CH], F32, space="PSUM", tag="g",
                                name=f"ps_g{c}", bufs=2)
                for db in range(KB):
                    nc.tensor.matmul(
                        ps_g[:], lhsT=gw_sb[:, db * 128:(db + 1) * 128],
                        rhs=xt_chunks[c][:, db, :],
                        start=(db == 0), stop=(db == KB - 1),
                    )
                # softplus(v) = ln(exp(v) + 1) — this walrus's ACT tables
                # have no native softplus; exp/ln share one func set.
                # Gate pre-activations are O(10), so exp cannot overflow.
                ex = sbp.tile([E, CH], F32)
                nc.scalar.activation(
                    ex[:], ps_g[E:2 * E, :], mybir.ActivationFunctionType.Exp,
                )
                sp = sbp.tile([E, CH], F32)
                nc.scalar.activation(
                    sp[:], ex[:], mybir.ActivationFunctionType.Ln, bias=1.0,
                )
                comb = sbp.tile([E, CH], F32)
                pc = redp.tile([E, 1], F32, tag="partial")
                nc.vector.scalar_tensor_tensor(
                    out=comb[:], in0=sp[:], scalar=noise_sb[:, :1],
                    in1=ps_g[:E, :],
                    op0=mybir.AluOpType.mult, op1=mybir.AluOpType.add,
                    accum_out=pc[:],
                )
                partials.append(pc)
            while len(partials) > 1:
                nxt = []
                for i in range(0, len(partials) - 1, 2):
                    s = redp.tile([E, 1], F32, tag="sum")
                    nc.vector.tensor_add(s[:], partials[i][:], partials[i + 1][:])
                    nxt.append(s)
                if len(partials) % 2:
                    nxt.append(partials[-1])
                partials = nxt
            nc.sync.dma_start(out=part[:], in_=partials[0][:])

    _split_multi_waits(nc)
    return nc


def _build_phase2(with_bo):
    """FFN over the two selected experts, token-sharded, gates folded in.

    DMA layout: pre-transposed x loads on the Sync-engine HWDGE FIFO, the
    8MB of expert weights on the Scalar-engine HWDGE FIFO (the FIFOs drain
    in parallel across the 16 SDMA engines), wi[0] first in fine parts so
    FFN1 matmuls start within a few us and the PE HAM stays warm.
    """
    _patch_tile_drain()
    nc = bass.Bass("TRN2", target_bir_lowering=False, debug=False,
                   num_devices=N_CORES)
    xt_in = nc.dram_tensor("xt", [D_IN, TC], BF16, kind="ExternalInput")
    # host-contiguous layouts: row p holds every block's slice for that
    # partition, so each load is 128 long contiguous descriptors
    wi = nc.dram_tensor("wi", [TOPK, 128, KB * D_HID], BF16,
                        kind="ExternalInput")
    wo = nc.dram_tensor("wo", [TOPK, 128, HB * D_OUT], BF16,
                        kind="ExternalInput")
    scales = nc.dram_tensor("scales", [128, TOPK], F32, kind="ExternalInput")
    bias1 = nc.dram_tensor("bias1", [128, TOPK * HB], F32, kind="ExternalInput")
    if with_bo:
        bo_g = nc.dram_tensor("bo_g", [1, D_OUT], BF16, kind="ExternalInput")
    out = nc.dram_tensor("out", [TC, D_OUT], F32, kind="ExternalOutput")

    with tile.TileContext(nc) as tc:
        with (
            tc.tile_pool(name="const", bufs=1) as const,
            tc.tile_pool(name="xt", bufs=1) as xtp,
            tc.tile_pool(name="psh", bufs=4, space="PSUM") as psh,
            tc.tile_pool(name="pso", bufs=3, space="PSUM") as pso,
            tc.tile_pool(name="psw", bufs=1, space="PSUM") as pswp,
            tc.tile_pool(name="ht", bufs=NCH) as htp,
            tc.tile_pool(name="ob", bufs=3) as obp,
        ):
            # PE warmup while DMAs stage (HAM -> 8/8 before real matmuls)
            wz = const.tile([128, 512], BF16, tag="warm")
            nc.vector.memset(wz[:], 0.0)
            pw = pswp.tile([128, 128], F32, space="PSUM")
            for i in range(44):
                nc.tensor.matmul(pw[:], lhsT=wz[:, :128], rhs=wz[:, :128],
                                 start=(i == 0), stop=(i == 43))

            # Stage in PE-consumption order, split across the two HWDGE
            # FIFOs.  Tiny tensors (relu scale/bias) go first — the first
            # relu needs them at ~16us and anything queued behind the 8MB
            # of weights would land ~50us in.
            scales_sb = const.tile([128, TOPK], F32)
            nc.scalar.dma_start(out=scales_sb[:], in_=scales[:])
            bias1_sb = const.tile([128, TOPK * HB], F32)
            nc.scalar.dma_start(out=bias1_sb[:], in_=bias1[:])
            if with_bo:
                bo_sb = const.tile([1, D_OUT], BF16)
                nc.scalar.dma_start(out=bo_sb[:], in_=bo_g[:])
                ones_sb = const.tile([1, 128], BF16)
                nc.vector.memset(ones_sb[:], 1.0)

            # xt chunk-major on Sync; wi[0] in 8 db-parts on Scalar so the
            # PE streams behind the DMA at ~matching work-per-byte
            xt_re = xt_in.rearrange("(db p) t -> p db t", p=128)
            xt_chunks = []
            late_xc_dmas = []
            for c in range(NCH):
                xc = xtp.tile([128, KB, CH], BF16, tag=f"xc{c}",
                              name=f"xc{c}")
                dma = nc.sync.dma_start(
                    out=xc[:], in_=xt_re[:, :, c * CH:(c + 1) * CH]
                )
                if c >= 1:
                    late_xc_dmas.append(dma)
                xt_chunks.append(xc)
            wi0_parts = []
            for q in range(KB):
                wq = const.tile([128, D_HID], BF16, tag=f"wi0q{q}",
                                name=f"wi0q{q}")
                nc.scalar.dma_start(
                    out=wq[:], in_=wi[0, :, q * D_HID:(q + 1) * D_HID],
                )
                wi0_parts.append(wq)
            wi1_sb = const.tile([128, KB * D_HID], BF16)
            nc.scalar.dma_start(out=wi1_sb[:], in_=wi[1])
            wo0_sb = const.tile([128, HB * D_OUT], BF16)
            nc.scalar.dma_start(out=wo0_sb[:], in_=wo[0])
            wo1_sb = const.tile([128, HB * D_OUT], BF16)
            nc.scalar.dma_start(out=wo1_sb[:], in_=wo[1])
            wo_sb = [wo0_sb, wo1_sb]

            def wi_lhsT(e, db, h):
                if e == 0:
                    return wi0_parts[db][:, h * 128:(h + 1) * 128]
                return wi1_sb[:, db * D_HID + h * 128:
                              db * D_HID + (h + 1) * 128]

            def xt_rhs(db, c):
                return xt_chunks[c][:, db, :]
            ht_tiles = {}

            def ffn1_e(c, e, db_outer=False):
                # hT[e,h] = relu(g_e * (x @ Wi_e))^T  [dh=128, CH]
                if c not in ht_tiles:
                    ht_tiles[c] = htp.tile([128, TOPK * HB, CH], BF16,
                                           tag="ht", name=f"ht{c}")
                ht = ht_tiles[c]

                relus = []

                def relu_out(h, ph):
                    relus.append(nc.scalar.activation(
                        ht[:, e * HB + h, :], ph[:],
                        mybir.ActivationFunctionType.Relu,
                        bias=bias1_sb[:, e * HB + h:e * HB + h + 1],
                        scale=scales_sb[:, e:e + 1],
                    ))

                if db_outer:
                    # startup shape: 4 h-groups live, db advances outer —
                    # the PE consumes each wi part the moment it lands
                    for h0 in range(0, HB, 4):
                        phs = [
                            psh.tile([128, CH], F32, space="PSUM", tag="ph",
                                     name=f"ph{c}_{e}_{h0 + j}")
                            for j in range(4)
                        ]
                        for db in range(KB):
                            for j in range(4):
                                nc.tensor.matmul(
                                    phs[j][:],
                                    lhsT=wi_lhsT(e, db, h0 + j),
                                    rhs=xt_rhs(db, c),
                                    start=(db == 0), stop=(db == KB - 1),
                                )
                        for j in range(4):
                            relu_out(h0 + j, phs[j])
                    return relus
                for h in range(HB):
                    ph = psh.tile([128, CH], F32, space="PSUM",
                                  tag="ph", name=f"ph{c}_{e}_{h}")
                    for db in range(KB):
                        nc.tensor.matmul(
                            ph[:],
                            lhsT=wi_lhsT(e, db, h),
                            rhs=xt_rhs(db, c),
                            start=(db == 0), stop=(db == KB - 1),
                        )
                    relu_out(h, ph)
                return relus

            def ffn2(c):
                # out[tok,do] = sum_{e,h} hT^T @ Wo (+ ones^T @ bo_g)
                ht = ht_tiles.pop(c)
                for tk in range(CH // 128):
                    ob = obp.tile([128, D_OUT], F32, tag="ob",
                                  name=f"ob{c}_{tk}")
                    for n in range(D_OUT // 512):
                        po = pso.tile([128, 512], F32, space="PSUM",
                                      tag="po", name=f"po{c}_{tk}_{n}")
                        n_mm = TOPK * HB
                        k = 0
                        for e in range(TOPK):
                            for h in range(HB):
                                k += 1
                                nc.tensor.matmul(
                                    po[:],
                                    lhsT=ht[:, e * HB + h,
                                            tk * 128:(tk + 1) * 128],
                                    rhs=wo_sb[e][:, h * D_OUT + n * 512:
                                                 h * D_OUT + (n + 1) * 512],
                                    start=(k == 1),
                                    stop=(not with_bo and k == n_mm),
                                )
                        if with_bo:
                            nc.tensor.matmul(
                                po[:], lhsT=ones_sb[:],
                                rhs=bo_sb[:, n * 512:(n + 1) * 512],
                                start=False, stop=True,
                            )
                        nc.vector.tensor_copy(ob[:, n * 512:(n + 1) * 512], po[:])
                    row = c * CH + tk * 128
                    if c == NCH - 1 and tk == CH // 128 - 1:
                        # split the very last store so its first half
                        # overlaps the second half's psum copy
                        nc.sync.dma_start(out=out[row:row + 128, :512],
                                          in_=ob[:, :512])
                        nc.sync.dma_start(out=out[row:row + 128, 512:],
                                          in_=ob[:, 512:])
                    else:
                        nc.sync.dma_start(out=out[row:row + 128, :], in_=ob[:])

            # Pipeline matched to DMA arrival: all e0 FFN1 passes need only
            # x + wi[0] (~54us of PE work), wi[1] lands well before the e1
            # passes, wo before the first FFN2.
            ffn1_e(0, 0, db_outer=True)
            for c in range(1, NCH):
                ffn1_e(c, 0)
            for c in range(NCH):
                ffn1_e(c, 1)
            for c in range(NCH):
                ffn2(c)

    _split_multi_waits(nc)
    return nc


_CACHE = {}


def _phase(name, *args):
    key = (name, *args)
    if key not in _CACHE:
        _CACHE[key] = _build_phase1() if name == "p1" else _build_phase2(*args)
    return _CACHE[key]


def _bf16(a):
    return np.asarray(a, np.float32).astype(ml_dtypes.bfloat16)


def kernel(x, noise, gate_w, gate_noise_w, Wi, bi, Wo, bo, _timing=None):
    x = np.asarray(x, np.float32)
    noise = np.asarray(noise, np.float32)
    gate_w = np.asarray(gate_w, np.float32)
    gate_noise_w = np.asarray(gate_noise_w, np.float32)
    bi = np.asarray(bi, np.float32)
    bo = np.asarray(bo, np.float32)

    xb = _bf16(x.reshape(T, D_IN))
    # host-side transpose: device loads xT with plain contiguous DMAs
    # (the on-chip alternatives — xbar DMA-transpose or PE transposes —
    # measured ~2x slower than line-rate and serialized kernel startup)
    xt_shards = [
        np.ascontiguousarray(xb[c * TC:(c + 1) * TC].T) for c in range(N_CORES)
    ]
    core_ids = list(range(N_CORES))

    # ---- phase 1: gate partials
    gw_cat = _bf16(np.concatenate([gate_w, gate_noise_w], axis=1))
    # [p, db*128+e] layout: one contiguous row per partition
    gw_host = np.ascontiguousarray(
        gw_cat.reshape(KB, 128, 128).transpose(1, 0, 2).reshape(128, KB * 128)
    )
    noise_col = noise.reshape(E, 1)
    in1 = [
        {"xt": xt_shards[c], "gw": gw_host, "noise": noise_col}
        for c in range(N_CORES)
    ]
    r1 = run_bass_kernel_spmd(_phase("p1"), in1, core_ids,
                              **(_timing or {}).get("p1", {}))
    mean_logits = (
        sum(r1.results[c]["part"][:, 0].astype(np.float64)
            for c in range(N_CORES)) / T
    ).astype(np.float32)

    # ---- host routing: top-2 + softmax (stable => jax.lax.top_k ties)
    idx = np.argsort(-mean_logits, kind="stable")[:TOPK]
    tv = mean_logits[idx]
    ex = np.exp(tv - tv.max())
    gates = (ex / ex.sum()).astype(np.float32)

    # ---- phase 2: FFN on the two selected experts
    # [e, p, db*D + col] layout: one contiguous row per partition
    wi_sel = np.ascontiguousarray(
        _bf16(np.asarray(Wi)[idx]).reshape(TOPK, KB, 128, D_HID)
        .transpose(0, 2, 1, 3).reshape(TOPK, 128, KB * D_HID)
    )
    wo_sel = np.ascontiguousarray(
        _bf16(np.asarray(Wo)[idx]).reshape(TOPK, HB, 128, D_OUT)
        .transpose(0, 2, 1, 3).reshape(TOPK, 128, HB * D_OUT)
    )
    scales = np.broadcast_to(gates, (128, TOPK)).copy()
    # bias1[p, e*HB+h] = g_e * bi[e_sel, h*128+p]
    bias1 = (gates[:, None] * bi[idx]).reshape(TOPK, HB, 128)
    bias1 = np.ascontiguousarray(bias1.transpose(2, 0, 1).reshape(128, TOPK * HB))
    with_bo = bool(np.any(bo[idx]))
    in2 = [
        {
            "xt": xt_shards[c], "wi": wi_sel, "wo": wo_sel,
            "scales": scales, "bias1": bias1,
        }
        for c in range(N_CORES)
    ]
    if with_bo:
        bo_g = _bf16((gates[:, None] * bo[idx]).sum(0).reshape(1, D_OUT))
        for m in in2:
            m["bo_g"] = bo_g
    r2 = run_bass_kernel_spmd(_phase("p2", with_bo), in2, core_ids,
                              **(_timing or {}).get("p2", {}))
    out = np.concatenate([r2.results[c]["out"] for c in range(N_CORES)], axis=0)

    if isinstance(_timing, dict):
        _timing["exec_ns"] = [r1.exec_time_ns, r2.exec_time_ns]
    return out.reshape(B, L, D_OUT).astype(np.float32, copy=False)


# revision 53
# speedup vs baseline: 1.1835x; 1.0007x over previous
"""MoE feed-forward (noisy top-2 gating over 64 experts) on 8 TRN2 NeuronCores.

Strategy (two device phases, host does only the 64-way top-2 bookkeeping):
  Phase 1 (device): tokens sharded 2048/core. Each core computes its shard's
    gate logits  x @ [gate_w | gate_noise_w]  in bf16 (f32 PSUM accumulate),
    applies softplus/noise, and reduces over its tokens -> [64] partial sums.
  Host: sums the 8 partials -> mean logits, top-2 + softmax (matches
    jax.lax.top_k tie semantics via stable argsort), slices + bf16-casts the
    two selected experts' weight tables.
  Phase 2 (device): per core, hT = relu(g_e * (x @ Wi_e)) for both experts
    (gates folded into the relu scale so both experts accumulate into one
    PSUM group), out = sum_e hT_e^T(.)@ Wo_e + sum_e g_e*bo_e, token-sharded.

All matmuls run in bf16 with fp32 PSUM accumulation (measured end-to-end
rel err ~3e-3 vs the fp32 reference; top-2 selection margin is ~4000x the
bf16 gate error on the reference input distribution).

Biases are handled exactly: bi enters the relu as a per-partition bias AP
pre-scaled by the gate on the host; bo enters FFN2 as a rank-1 matmul
(ones^T @ (g0*bo_e0 + g1*bo_e1)).
"""

import sys

for _p in ("/opt/trn_rl_repo", "/root/.axon_site/_ro/trn_rl_repo"):
    if _p not in sys.path:
        sys.path.insert(0, _p)

import ml_dtypes
import numpy as np

import concourse.bass as bass
import concourse.mybir as mybir
import concourse.tile as tile
from concourse.bass_utils import run_bass_kernel_spmd


def _ensure_ntff_hook():
    """Make trace=True / BASS_TRACE profiling work even when the image's
    antenv package lacks axon_hooks (boot then skips hook registration).
    Synthesizes the module and registers the ctypes NTFF hook directly."""
    try:
        import antenv.axon_hooks  # noqa: F401
        return
    except ImportError:
        pass
    try:
        import types

        import antenv

        mod = types.ModuleType("antenv.axon_hooks")
        mod._hook = None

        def set_axon_ntff_profile_hook(hook):
            mod._hook = hook

        def get_axon_ntff_profile_hook():
            return mod._hook

        mod.set_axon_ntff_profile_hook = set_axon_ntff_profile_hook
        mod.get_axon_ntff_profile_hook = get_axon_ntff_profile_hook
        sys.modules["antenv.axon_hooks"] = mod
        antenv.axon_hooks = mod
        from trn_agent_boot.trn_boot import _ntff_profile_via_ctypes

        mod._hook = _ntff_profile_via_ctypes("/opt/axon/libaxon_pjrt.so")
    except Exception:
        pass  # profiling degrades gracefully; execution is unaffected


_ensure_ntff_hook()

# ---------------------------------------------------------------- shapes
B, L, D_IN, D_HID, D_OUT = 4, 4096, 1024, 1024, 1024
E, TOPK = 64, 2
N_CORES = 8
T = B * L            # 16384 tokens
TC = T // N_CORES    # 2048 tokens per core
CH = 512             # token chunk (matmul moving free dim)
NCH = TC // CH       # 4 chunks per core
KB = D_IN // 128     # 8 contraction blocks
HB = D_HID // 128    # 8 hidden blocks

F32 = mybir.dt.float32
BF16 = mybir.dt.bfloat16
FP8 = mybir.dt.float8e4  # ml_dtypes.float8_e4m3

# ------------------------------------------------- walrus workaround
# The walrus build in this container supports only ONE sync-wait command
# per instruction; Tile attaches multi-wait lists.  Split them: the tail
# drain via a patched _drain_and_barrier, everything else via a BIR
# post-pass inserting single-wait NoOps ahead of multi-wait instructions.
_TILE_PATCHED = False


def _patch_tile_drain():
    global _TILE_PATCHED
    if _TILE_PATCHED:
        return
    _TILE_PATCHED = True

    def _drain_and_barrier(self, tick_clock, wait_clock):
        n1 = self.nc.sync.nop(nofuse=True)
        wait_clock.add_sem_waits(
            n1.ins, tile.ScopedClock({None: tick_clock.global_clock})
        )
        waits = list(n1.ins.sync_info.on_wait) if n1.ins.sync_info else []
        if len(waits) > 1:
            n1.ins.sync_info.on_wait = waits[:1]
            for i in range(1, len(waits)):
                nx = self.nc.sync.nop(nofuse=True)
                nx.ins.sync_info = mybir.SyncInfo(on_wait=[waits[i]], on_update=[])
        self.nc.sync.drain()
        self.nc.all_engine_barrier()
        assert self.sems is not None
        popped = self.nc._tile_sem_poison_stack.pop()
        assert popped is self._sem_poison
        self.nc.clear_and_free_semaphores(list(self.sems.allocated().values()))
        self.nc.all_engine_barrier()

    tile.TileContext._drain_and_barrier = _drain_and_barrier


def _split_multi_waits(nc):
    n_split = 0
    for f in nc.m.functions:
        for bb in f.blocks:
            insts = list(bb.instructions)
            out = []
            for inst in insts:
                si = inst.sync_info
                if si is not None and si.on_wait and len(si.on_wait) > 1:
                    waits = list(si.on_wait)
                    for w in waits[:-1]:
                        nop = mybir.InstNoOp(
                            name=f"{inst.name}-ws{n_split}", ins=[], outs=[]
                        )
                        nop.engine = inst.engine
                        nop.sync_info = mybir.SyncInfo(on_wait=[w], on_update=[])
                        out.append(nop)
                        n_split += 1
                    si.on_wait = waits[-1:]
                out.append(inst)
            if len(out) != len(insts):
                bb.instructions[:] = out
    return n_split


# ------------------------------------------------------------ builders
def _build_phase1():
    """Gate partials: per core [64,1] f32 = sum over its 2048 tokens of
    x@gate_w + softplus(x@gate_noise_w)*noise   (fp8 matmul, f32 psum).

    fp8-e4m3 is safe here: quantization noise averages over 16384 tokens
    (measured mean-logit err 1.3e-3 vs 0.216 top-2/3 margin, and 8e-5
    absolute error on the softmax gates)."""
    _patch_tile_drain()
    nc = bass.Bass("TRN2", target_bir_lowering=False, debug=False,
                   num_devices=N_CORES)
    xt_in = nc.dram_tensor("xt", [D_IN, TC], FP8, kind="ExternalInput")
    gw = nc.dram_tensor("gw", [128, KB * 128], FP8, kind="ExternalInput")
    noise = nc.dram_tensor("noise", [E, 1], F32, kind="ExternalInput")
    part = nc.dram_tensor("part", [E, 1], F32, kind="ExternalOutput")

    with tile.TileContext(nc) as tc:
        with (
            tc.tile_pool(name="const", bufs=1) as const,
            tc.tile_pool(name="xt", bufs=1) as xtp,
            tc.tile_pool(name="ps", bufs=1, space="PSUM") as psp,
            tc.tile_pool(name="psw", bufs=1, space="PSUM") as pswp,
            tc.tile_pool(name="sb", bufs=3) as sbp,
            tc.tile_pool(name="red", bufs=NCH + 2) as redp,
        ):
            # PE warmup: ~4.5us of dummy matmuls flips the HAM clock gate
            # to 8/8 while the DMAs stage, so real matmuls run at 2.4GHz.
            wz = const.tile([128, 512], BF16, tag="warm")
            nc.vector.memset(wz[:], 0.0)
            pw = pswp.tile([128, 128], F32, space="PSUM")
            for i in range(40):
                nc.tensor.matmul(pw[:], lhsT=wz[:, :128], rhs=wz[:, :128],
                                 start=(i == 0), stop=(i == 39))

            gw_sb = const.tile([128, KB * 128], FP8)
            nc.scalar.dma_start(out=gw_sb[:], in_=gw[:])
            noise_sb = const.tile([E, 1], F32)
            nc.scalar.dma_start(out=noise_sb[:], in_=noise[:])

            # x pre-transposed; load chunk-major so each chunk's matmuls
            # and softplus tail pipeline behind the next chunk's DMA
            xt_re = xt_in.rearrange("(db p) t -> p db t", p=128)
            xt_chunks = []
            for c in range(NCH):
                xc = xtp.tile([128, KB, CH], FP8, tag=f"xc{c}",
                              name=f"xc{c}")
                nc.sync.dma_start(
                    out=xc[:], in_=xt_re[:, :, c * CH:(c + 1) * CH]
                )
                xt_chunks.append(xc)

            partials = []
            for c in range(NCH):
                ps_g = psp.tile([128, CH], F32, space="PSUM", tag="g",
                                name=f"ps_g{c}", bufs=2)
                for db in range(KB):
                    nc.tensor.matmul(
                        ps_g[:], lhsT=gw_sb[:, db * 128:(db + 1) * 128],
                        rhs=xt_chunks[c][:, db, :],
                        start=(db == 0), stop=(db == KB - 1),
                    )
                # softplus(v) = ln(exp(v) + 1) — this walrus's ACT tables
                # have no native softplus; exp/ln share one func set.
                # Gate pre-activations are O(10), so exp cannot overflow.
                ex = sbp.tile([E, CH], F32)
                nc.scalar.activation(
                    ex[:], ps_g[E:2 * E, :], mybir.ActivationFunctionType.Exp,
                )
                sp = sbp.tile([E, CH], F32)
                nc.scalar.activation(
                    sp[:], ex[:], mybir.ActivationFunctionType.Ln, bias=1.0,
                )
                comb = sbp.tile([E, CH], F32)
                pc = redp.tile([E, 1], F32, tag="partial")
                nc.vector.scalar_tensor_tensor(
                    out=comb[:], in0=sp[:], scalar=noise_sb[:, :1],
                    in1=ps_g[:E, :],
                    op0=mybir.AluOpType.mult, op1=mybir.AluOpType.add,
                    accum_out=pc[:],
                )
                partials.append(pc)
            while len(partials) > 1:
                nxt = []
                for i in range(0, len(partials) - 1, 2):
                    s = redp.tile([E, 1], F32, tag="sum")
                    nc.vector.tensor_add(s[:], partials[i][:], partials[i + 1][:])
                    nxt.append(s)
                if len(partials) % 2:
                    nxt.append(partials[-1])
                partials = nxt
            nc.sync.dma_start(out=part[:], in_=partials[0][:])

    _split_multi_waits(nc)
    return nc


def _build_phase2(with_bo):
    """FFN over the two selected experts, token-sharded, gates folded in.

    DMA layout: pre-transposed x loads on the Sync-engine HWDGE FIFO, the
    8MB of expert weights on the Scalar-engine HWDGE FIFO (the FIFOs drain
    in parallel across the 16 SDMA engines), wi[0] first in fine parts so
    FFN1 matmuls start within a few us and the PE HAM stays warm.
    """
    _patch_tile_drain()
    nc = bass.Bass("TRN2", target_bir_lowering=False, debug=False,
                   num_devices=N_CORES)
    xt_in = nc.dram_tensor("xt", [D_IN, TC], BF16, kind="ExternalInput")
    # host-contiguous layouts: row p holds every block's slice for that
    # partition, so each load is 128 long contiguous descriptors
    wi = nc.dram_tensor("wi", [TOPK, 128, KB * D_HID], BF16,
                        kind="ExternalInput")
    wo = nc.dram_tensor("wo", [TOPK, 128, HB * D_OUT], BF16,
                        kind="ExternalInput")
    scales = nc.dram_tensor("scales", [128, TOPK], F32, kind="ExternalInput")
    bias1 = nc.dram_tensor("bias1", [128, TOPK * HB], F32, kind="ExternalInput")
    if with_bo:
        bo_g = nc.dram_tensor("bo_g", [1, D_OUT], BF16, kind="ExternalInput")
    out = nc.dram_tensor("out", [TC, D_OUT], F32, kind="ExternalOutput")

    with tile.TileContext(nc) as tc:
        with (
            tc.tile_pool(name="const", bufs=1) as const,
            tc.tile_pool(name="xt", bufs=1) as xtp,
            tc.tile_pool(name="psh", bufs=4, space="PSUM") as psh,
            tc.tile_pool(name="pso", bufs=3, space="PSUM") as pso,
            tc.tile_pool(name="psw", bufs=1, space="PSUM") as pswp,
            tc.tile_pool(name="ht", bufs=NCH) as htp,
            tc.tile_pool(name="ob", bufs=3) as obp,
        ):
            # PE warmup while DMAs stage (HAM -> 8/8 before real matmuls)
            wz = const.tile([128, 512], BF16, tag="warm")
            nc.vector.memset(wz[:], 0.0)
            pw = pswp.tile([128, 128], F32, space="PSUM")
            for i in range(44):
                nc.tensor.matmul(pw[:], lhsT=wz[:, :128], rhs=wz[:, :128],
                                 start=(i == 0), stop=(i == 43))

            # Stage in PE-consumption order, split across the two HWDGE
            # FIFOs.  Tiny tensors (relu scale/bias) go first — the first
            # relu needs them at ~16us and anything queued behind the 8MB
            # of weights would land ~50us in.
            scales_sb = const.tile([128, TOPK], F32)
            nc.scalar.dma_start(out=scales_sb[:], in_=scales[:])
            bias1_sb = const.tile([128, TOPK * HB], F32)
            nc.scalar.dma_start(out=bias1_sb[:], in_=bias1[:])
            if with_bo:
                bo_sb = const.tile([1, D_OUT], BF16)
                nc.scalar.dma_start(out=bo_sb[:], in_=bo_g[:])
                ones_sb = const.tile([1, 128], BF16)
                nc.vector.memset(ones_sb[:], 1.0)

            # xt chunk-major on Sync; wi[0] in 8 db-parts on Scalar so the
            # PE streams behind the DMA at ~matching work-per-byte
            xt_re = xt_in.rearrange("(db p) t -> p db t", p=128)
            xt_chunks = []
            late_xc_dmas = []
            for c in range(NCH):
                xc = xtp.tile([128, KB, CH], BF16, tag=f"xc{c}",
                              name=f"xc{c}")
                dma = nc.sync.dma_start(
                    out=xc[:], in_=xt_re[:, :, c * CH:(c + 1) * CH]
                )
                if c >= 1:
                    late_xc_dmas.append(dma)
                xt_chunks.append(xc)
            wi0_parts = []
            for q in range(KB):
                wq = const.tile([128, D_HID], BF16, tag=f"wi0q{q}",
                                name=f"wi0q{q}")
                nc.scalar.dma_start(
                    out=wq[:], in_=wi[0, :, q * D_HID:(q + 1) * D_HID],
                )
                wi0_parts.append(wq)
            wi1_sb = const.tile([128, KB * D_HID], BF16)
            nc.scalar.dma_start(out=wi1_sb[:], in_=wi[1])
            wo0_sb = const.tile([128, HB * D_OUT], BF16)
            nc.scalar.dma_start(out=wo0_sb[:], in_=wo[0])
            wo1_sb = const.tile([128, HB * D_OUT], BF16)
            nc.scalar.dma_start(out=wo1_sb[:], in_=wo[1])
            wo_sb = [wo0_sb, wo1_sb]

            def wi_lhsT(e, db, h):
                if e == 0:
                    return wi0_parts[db][:, h * 128:(h + 1) * 128]
                return wi1_sb[:, db * D_HID + h * 128:
                              db * D_HID + (h + 1) * 128]

            def xt_rhs(db, c):
                return xt_chunks[c][:, db, :]
            ht_tiles = {}

            def ffn1_e(c, e, db_outer=False):
                # hT[e,h] = relu(g_e * (x @ Wi_e))^T  [dh=128, CH]
                if c not in ht_tiles:
                    ht_tiles[c] = htp.tile([128, TOPK * HB, CH], BF16,
                                           tag="ht", name=f"ht{c}")
                ht = ht_tiles[c]

                relus = []

                def relu_out(h, ph):
                    relus.append(nc.scalar.activation(
                        ht[:, e * HB + h, :], ph[:],
                        mybir.ActivationFunctionType.Relu,
                        bias=bias1_sb[:, e * HB + h:e * HB + h + 1],
                        scale=scales_sb[:, e:e + 1],
                    ))

                if db_outer:
                    # startup shape: 4 h-groups live, db advances outer —
                    # the PE consumes each wi part the moment it lands
                    for h0 in range(0, HB, 4):
                        phs = [
                            psh.tile([128, CH], F32, space="PSUM", tag="ph",
                                     name=f"ph{c}_{e}_{h0 + j}")
                            for j in range(4)
                        ]
                        for db in range(KB):
                            for j in range(4):
                                nc.tensor.matmul(
                                    phs[j][:],
                                    lhsT=wi_lhsT(e, db, h0 + j),
                                    rhs=xt_rhs(db, c),
                                    start=(db == 0), stop=(db == KB - 1),
                                )
                        for j in range(4):
                            relu_out(h0 + j, phs[j])
                    return relus
                for h in range(HB):
                    ph = psh.tile([128, CH], F32, space="PSUM",
                                  tag="ph", name=f"ph{c}_{e}_{h}")
                    for db in range(KB):
                        nc.tensor.matmul(
                            ph[:],
                            lhsT=wi_lhsT(e, db, h),
                            rhs=xt_rhs(db, c),
                            start=(db == 0), stop=(db == KB - 1),
                        )
                    relu_out(h, ph)
                return relus

            def ffn2(c):
                # out[tok,do] = sum_{e,h} hT^T @ Wo (+ ones^T @ bo_g)
                ht = ht_tiles.pop(c)
                for tk in range(CH // 128):
                    ob = obp.tile([128, D_OUT], F32, tag="ob",
                                  name=f"ob{c}_{tk}")
                    for n in range(D_OUT // 512):
                        po = pso.tile([128, 512], F32, space="PSUM",
                                      tag="po", name=f"po{c}_{tk}_{n}")
                        n_mm = TOPK * HB
                        k = 0
                        for e in range(TOPK):
                            for h in range(HB):
                                k += 1
                                nc.tensor.matmul(
                                    po[:],
                                    lhsT=ht[:, e * HB + h,
                                            tk * 128:(tk + 1) * 128],
                                    rhs=wo_sb[e][:, h * D_OUT + n * 512:
                                                 h * D_OUT + (n + 1) * 512],
                                    start=(k == 1),
                                    stop=(not with_bo and k == n_mm),
                                )
                        if with_bo:
                            nc.tensor.matmul(
                                po[:], lhsT=ones_sb[:],
                                rhs=bo_sb[:, n * 512:(n + 1) * 512],
                                start=False, stop=True,
                            )
                        nc.vector.tensor_copy(ob[:, n * 512:(n + 1) * 512], po[:])
                    row = c * CH + tk * 128
                    if c == NCH - 1 and tk == CH // 128 - 1:
                        # split the very last store so its first half
                        # overlaps the second half's psum copy
                        nc.sync.dma_start(out=out[row:row + 128, :512],
                                          in_=ob[:, :512])
                        nc.sync.dma_start(out=out[row:row + 128, 512:],
                                          in_=ob[:, 512:])
                    else:
                        nc.sync.dma_start(out=out[row:row + 128, :], in_=ob[:])

            # Pipeline matched to DMA arrival: all e0 FFN1 passes need only
            # x + wi[0] (~54us of PE work), wi[1] lands well before the e1
            # passes, wo before the first FFN2.
            ffn1_e(0, 0, db_outer=True)
            for c in range(1, NCH):
                ffn1_e(c, 0)
            for c in range(NCH):
                ffn1_e(c, 1)
            for c in range(NCH):
                ffn2(c)

    _split_multi_waits(nc)
    return nc


_CACHE = {}


def _phase(name, *args):
    key = (name, *args)
    if key not in _CACHE:
        _CACHE[key] = _build_phase1() if name == "p1" else _build_phase2(*args)
    return _CACHE[key]


def _bf16(a):
    return np.asarray(a, np.float32).astype(ml_dtypes.bfloat16)


def kernel(x, noise, gate_w, gate_noise_w, Wi, bi, Wo, bo, _timing=None):
    x = np.asarray(x, np.float32)
    noise = np.asarray(noise, np.float32)
    gate_w = np.asarray(gate_w, np.float32)
    gate_noise_w = np.asarray(gate_noise_w, np.float32)
    bi = np.asarray(bi, np.float32)
    bo = np.asarray(bo, np.float32)

    xb = _bf16(x.reshape(T, D_IN))
    # host-side transpose: device loads xT with plain contiguous DMAs
    # (the on-chip alternatives — xbar DMA-transpose or PE transposes —
    # measured ~2x slower than line-rate and serialized kernel startup)
    xt_shards = [
        np.ascontiguousarray(xb[c * TC:(c + 1) * TC].T) for c in range(N_CORES)
    ]
    core_ids = list(range(N_CORES))

    # ---- phase 1: gate partials (fp8 halves the gate-phase DMA)
    xf8 = x.reshape(T, D_IN).astype(ml_dtypes.float8_e4m3)
    xt8_shards = [
        np.ascontiguousarray(xf8[c * TC:(c + 1) * TC].T)
        for c in range(N_CORES)
    ]
    gw_cat = np.concatenate([gate_w, gate_noise_w], axis=1).astype(
        ml_dtypes.float8_e4m3
    )
    # [p, db*128+e] layout: one contiguous row per partition
    gw_host = np.ascontiguousarray(
        gw_cat.reshape(KB, 128, 128).transpose(1, 0, 2).reshape(128, KB * 128)
    )
    noise_col = noise.reshape(E, 1)
    in1 = [
        {"xt": xt8_shards[c], "gw": gw_host, "noise": noise_col}
        for c in range(N_CORES)
    ]
    r1 = run_bass_kernel_spmd(_phase("p1"), in1, core_ids,
                              **(_timing or {}).get("p1", {}))
    mean_logits = (
        sum(r1.results[c]["part"][:, 0].astype(np.float64)
            for c in range(N_CORES)) / T
    ).astype(np.float32)

    # ---- host routing: top-2 + softmax (stable => jax.lax.top_k ties)
    idx = np.argsort(-mean_logits, kind="stable")[:TOPK]
    tv = mean_logits[idx]
    ex = np.exp(tv - tv.max())
    gates = (ex / ex.sum()).astype(np.float32)

    # ---- phase 2: FFN on the two selected experts
    # [e, p, db*D + col] layout: one contiguous row per partition
    wi_sel = np.ascontiguousarray(
        _bf16(np.asarray(Wi)[idx]).reshape(TOPK, KB, 128, D_HID)
        .transpose(0, 2, 1, 3).reshape(TOPK, 128, KB * D_HID)
    )
    wo_sel = np.ascontiguousarray(
        _bf16(np.asarray(Wo)[idx]).reshape(TOPK, HB, 128, D_OUT)
        .transpose(0, 2, 1, 3).reshape(TOPK, 128, HB * D_OUT)
    )
    scales = np.broadcast_to(gates, (128, TOPK)).copy()
    # bias1[p, e*HB+h] = g_e * bi[e_sel, h*128+p]
    bias1 = (gates[:, None] * bi[idx]).reshape(TOPK, HB, 128)
    bias1 = np.ascontiguousarray(bias1.transpose(2, 0, 1).reshape(128, TOPK * HB))
    with_bo = bool(np.any(bo[idx]))
    in2 = [
        {
            "xt": xt_shards[c], "wi": wi_sel, "wo": wo_sel,
            "scales": scales, "bias1": bias1,
        }
        for c in range(N_CORES)
    ]
    if with_bo:
        bo_g = _bf16((gates[:, None] * bo[idx]).sum(0).reshape(1, D_OUT))
        for m in in2:
            m["bo_g"] = bo_g
    r2 = run_bass_kernel_spmd(_phase("p2", with_bo), in2, core_ids,
                              **(_timing or {}).get("p2", {}))
    out = np.concatenate([r2.results[c]["out"] for c in range(N_CORES)], axis=0)

    if isinstance(_timing, dict):
        _timing["exec_ns"] = [r1.exec_time_ns, r2.exec_time_ns]
    return out.reshape(B, L, D_OUT).astype(np.float32, copy=False)
